# revision 22
# baseline (speedup 1.0000x reference)
"""Fused multi-head attention block (qkv proj + RoPE + SDPA + out proj) on 8
Trainium2 NeuronCores.

Sharding: data-parallel over batch (4) x tensor-parallel over heads (2 groups
of 8). Core c handles batch c//2, head group c%2. Each core returns a partial
(2048, 1024) output; the host sums the two head-group partials per batch.

v3 design: PE and ACT co-bottleneck (~280us each), everything else hidden.
  - all matmuls bf16; e-paired S matmuls (row groups 0-1 / 2-3) run
    concurrently in the PE array
  - every PSUM allocation is a [128,1024]-shaped 2-bank tile from one pool
    (proj chunk pairs, v tm-pairs, S tiles, c_proj qm units) + 2 ya tiles:
    4 + 4 = 8 banks exactly
  - softmax denominators via a ones-column in v (M=65 Y matmuls)
  - pass-boundary: ya is copied to SBUF immediately so normalization never
    stalls the next pass (keeps HAM warm)
  - strips 0-1 emitted up front; strips 2-3 and lo=1 projections pumped into
    the first SDPA pass; qh=0 c_proj pumped into qh=1 passes
  - scalar engine does nothing but the 256 exps

Self-contained: hardcodes B=4, T=2048, C=1024, H=16, D=64.
"""

import numpy as np

B, T, C = 4, 2048, 1024
H, D = 16, 64
HL = H // 2            # heads per core
N_CORES = 8
ROPE_THETA = 10000.0
NSTRIP = 4
SW = T // NSTRIP       # 512

_NC = None


def _build_nc():
    import concourse.mybir as mybir
    import concourse.tile as tile
    from concourse import bacc

    F32 = mybir.dt.float32
    BF16 = mybir.dt.bfloat16
    EXP = mybir.ActivationFunctionType.Exp

    nc = bacc.Bacc("TRN2", target_bir_lowering=False, debug=False, num_devices=N_CORES)

    xt = nc.dram_tensor("xt", [C, T], F32, kind="ExternalInput")        # x[b].T
    wq = nc.dram_tensor("wq", [C, 512], F32, kind="ExternalInput")      # [C, f']
    wk = nc.dram_tensor("wk", [C, 512], F32, kind="ExternalInput")
    wv = nc.dram_tensor("wv", [C, 512], F32, kind="ExternalInput")      # [C, h*64+d]
    wp = nc.dram_tensor("wp", [512, C], F32, kind="ExternalInput")      # [h*64+d, o]
    cost = nc.dram_tensor("cost", [128, T], F32, kind="ExternalInput")
    sint = nc.dram_tensor("sint", [128, T], F32, kind="ExternalInput")
    out = nc.dram_tensor("out", [T, C], F32, kind="ExternalOutput")

    with tile.TileContext(nc) as tc:
        with (
            tc.tile_pool(name="res", bufs=1) as res,            # resident tensors
            tc.tile_pool(name="stg", bufs=2) as stg,            # fp32 DMA staging
            tc.tile_pool(name="rt", bufs=4) as rt,              # rope temporaries
            tc.tile_pool(name="ptp", bufs=4) as ptp,            # exp outputs
            tc.tile_pool(name="nrm", bufs=2) as nrm,            # normalization
            tc.tile_pool(name="obp", bufs=2) as obp,            # output staging
            tc.tile_pool(name="psS", bufs=2, space="PSUM") as psS,   # [128,1024] x2
            tc.tile_pool(name="psY", bufs=1, space="PSUM") as psY,   # ya0/ya1
        ):
            # ---- resident allocations ----
            wqs = [res.tile([128, 512], BF16, name=f"wqs{k}", tag=f"wqs{k}") for k in range(8)]
            wks = [res.tile([128, 512], BF16, name=f"wks{k}", tag=f"wks{k}") for k in range(8)]
            wvs = [res.tile([128, 512], BF16, name=f"wvs{k}", tag=f"wvs{k}") for k in range(8)]
            wps = [res.tile([128, 1024], BF16, name=f"wps{c}", tag=f"wps{c}") for c in range(4)]
            ct = res.tile([128, T], F32, name="ct", tag="ct")
            st = res.tile([128, T], F32, name="st", tag="st")
            qbf = [res.tile([128, T], BF16, name=f"qbf{j}", tag=f"qbf{j}") for j in range(4)]
            kbf = [res.tile([128, T], BF16, name=f"kbf{j}", tag=f"kbf{j}") for j in range(4)]
            vbf = [res.tile([128, 520], BF16, name=f"vbf{t}", tag=f"vbf{t}") for t in range(16)]
            ytf = [res.tile([128, T], BF16, name=f"ytf{j}", tag=f"ytf{j}") for j in range(4)]
            xts = [
                [res.tile([128, SW], BF16, name=f"x{s}_{k}", tag=f"x{s}_{k}") for k in range(8)]
                for s in range(NSTRIP)
            ]

            nc.sync.dma_start(ct[:], cost[:])
            nc.sync.dma_start(st[:], sint[:])

            def load_w(dram, dst, k):
                wstg = stg.tile([128, 512], F32, name=f"wstg_{dst[k].name}", tag="wstg")
                nc.sync.dma_start(wstg[:], dram[k * 128:(k + 1) * 128, :])
                nc.vector.tensor_copy(dst[k][:], wstg[:])

            def emit_xdma(s):
                for k in range(8):
                    xstg = stg.tile([128, SW], F32, name=f"xstg{s}_{k}", tag="xstg")
                    nc.sync.dma_start(xstg[:], xt[k * 128:(k + 1) * 128, s * SW:(s + 1) * SW])
                    nc.vector.tensor_copy(xts[s][k][:], xstg[:])

            def emit_vpair(s, tp):
                # v projection for t-chunks (s*4 + 2*tp, +1) -> one 2-bank tile
                vps = psS.tile([128, 1024], F32, name=f"vps{s}_{tp}", tag="stt")
                for half in range(2):
                    tm = 2 * tp + half
                    for k in range(8):
                        nc.tensor.matmul(
                            vps[:, half * 512:(half + 1) * 512],
                            xts[s][k][:, tm * 128:(tm + 1) * 128],
                            wvs[k][:],
                            start=(k == 0),
                            stop=(k == 7),
                        )
                    t = s * 4 + tm
                    va = vbf[t][:].rearrange("p (h x) -> p h x", x=65)
                    nc.vector.tensor_copy(
                        va[:, :, 0:64],
                        vps[:, half * 512:(half + 1) * 512].rearrange("p (h d) -> p h d", d=64),
                    )
                    nc.vector.memset(va[:, :, 64], 1.0)

            def emit_qk_pair(wts, dst, lo, s, nm):
                # chunks (lo, lo+2) of q/k for strip s into one 2-bank tile,
                # then rope into dst (qbf or kbf)
                ps = psS.tile([128, 1024], F32, name=f"ps_{nm}", tag="stt")
                for half, c in ((0, lo), (1, 2 + lo)):
                    for k in range(8):
                        nc.tensor.matmul(
                            ps[:, half * 512:(half + 1) * 512],
                            wts[k][:, c * 128:(c + 1) * 128],
                            xts[s][k][:],
                            start=(k == 0),
                            stop=(k == 7),
                        )
                ps1 = ps[:, 0:512]
                ps3 = ps[:, 512:1024]
                cs = ct[:, s * SW:(s + 1) * SW]
                sn = st[:, s * SW:(s + 1) * SW]
                a = rt.tile([128, SW], BF16, name=f"ra_{nm}", tag="rt")
                nc.vector.tensor_mul(a[:], ps1, cs)
                c2 = rt.tile([128, SW], BF16, name=f"rc_{nm}", tag="rt")
                nc.vector.tensor_mul(c2[:], ps1, sn)
                b = rt.tile([128, SW], BF16, name=f"rb_{nm}", tag="rt")
                nc.vector.tensor_mul(b[:], ps3, sn)
                d = rt.tile([128, SW], BF16, name=f"rd_{nm}", tag="rt")
                nc.vector.tensor_mul(d[:], ps3, cs)
                for hh in range(4):
                    h = lo * 4 + hh
                    w, j, e = hh * 32, h // 2, h % 2
                    nc.vector.tensor_sub(
                        dst[j][e * 64:e * 64 + 32, s * SW:(s + 1) * SW],
                        a[w:w + 32, :], b[w:w + 32, :],
                    )
                    nc.vector.tensor_add(
                        dst[j][e * 64 + 32:e * 64 + 64, s * SW:(s + 1) * SW],
                        c2[w:w + 32, :], d[w:w + 32, :],
                    )

            def emit_cproj_qm(qm):
                cp = psS.tile([128, 1024], F32, name=f"cp{qm}", tag="stt")
                for oh in range(2):
                    for c in range(4):
                        nc.tensor.matmul(
                            cp[:, oh * 512:(oh + 1) * 512],
                            ytf[c][:, qm * 128:(qm + 1) * 128],
                            wps[c][:, oh * 512:(oh + 1) * 512],
                            start=(c == 0), stop=(c == 3),
                        )
                ob = obp.tile([128, 1024], F32, name=f"ob{qm}", tag="ob")
                nc.vector.tensor_copy(ob[:], cp[:])
                nc.sync.dma_start(out[qm * 128:(qm + 1) * 128, :], ob[:])

            def load_wp():
                for c in range(4):
                    for half in range(2):
                        wpstg = stg.tile([128, 512], F32, name=f"wpstg{c}_{half}", tag="wstg")
                        nc.sync.dma_start(
                            wpstg[:], wp[c * 128:(c + 1) * 128, half * 512:(half + 1) * 512]
                        )
                        nc.vector.tensor_copy(
                            wps[c][:, half * 512:(half + 1) * 512], wpstg[:]
                        )

            # ---- stage 1 lead-in: strips 0,1 ----
            emit_xdma(0)
            for k in range(8):
                load_w(wq, wqs, k)
                load_w(wk, wks, k)
            emit_qk_pair(wqs, qbf, 0, 0, "q0s0")
            emit_qk_pair(wks, kbf, 0, 0, "k0s0")
            for k in range(8):
                load_w(wv, wvs, k)
            emit_vpair(0, 0)
            emit_vpair(0, 1)
            emit_xdma(1)
            emit_qk_pair(wqs, qbf, 0, 1, "q0s1")
            emit_qk_pair(wks, kbf, 0, 1, "k0s1")
            emit_vpair(1, 0)
            emit_vpair(1, 1)
            emit_xdma(2)
            emit_xdma(3)
            load_wp()

            # deferred stage-1 work, pumped into SDPA pass (qh0, j0)
            deferred = []
            for s in (2, 3):
                deferred.append(lambda s=s: emit_qk_pair(wqs, qbf, 0, s, f"q0s{s}"))
                deferred.append(lambda s=s: emit_qk_pair(wks, kbf, 0, s, f"k0s{s}"))
                deferred.append(lambda s=s: emit_vpair(s, 0))
                deferred.append(lambda s=s: emit_vpair(s, 1))
            for s in range(NSTRIP):
                deferred.append(lambda s=s: emit_qk_pair(wqs, qbf, 1, s, f"q1s{s}"))
                deferred.append(lambda s=s: emit_qk_pair(wks, kbf, 1, s, f"k1s{s}"))

            def pump():
                if deferred:
                    deferred.pop(0)()

            cproj_work = []

            def pump_cproj():
                if cproj_work:
                    emit_cproj_qm(cproj_work.pop(0))

            # ---- stage 2: SDPA, e-paired, qh-outer ----
            for qh in range(2):
                for j in range(4):
                    if qh == 0 and j == 2:
                        while deferred:
                            pump()
                    ya = [
                        psY.tile([65, 1024], F32, name=f"ya_j{j}h{qh}e{e}", tag=f"ya{e}")
                        for e in range(2)
                    ]
                    for kc in range(16):
                        stt = []
                        for e in range(2):
                            s_e = psS.tile(
                                [128, 1024], F32, name=f"st_j{j}h{qh}e{e}k{kc}", tag="stt"
                            )
                            for qs in range(2):
                                q0 = qh * 1024 + qs * 512
                                nc.tensor.matmul(
                                    s_e[:, qs * 512:(qs + 1) * 512],
                                    kbf[j][e * 64:e * 64 + 64, kc * 128:(kc + 1) * 128],
                                    qbf[j][e * 64:e * 64 + 64, q0:q0 + 512],
                                    start=True, stop=True,
                                    tile_position=(e * 64, 0),
                                )
                            stt.append(s_e)
                        for e in range(2):
                            h = 2 * j + e
                            pt = ptp.tile(
                                [128, 1024], BF16, name=f"pt_j{j}h{qh}e{e}k{kc}", tag="pt"
                            )
                            nc.scalar.activation(pt[:], stt[e][:], EXP, scale=0.125)
                            for qs in range(2):
                                nc.tensor.matmul(
                                    ya[e][:, qs * 512:(qs + 1) * 512],
                                    vbf[kc][:, h * 65:(h + 1) * 65],
                                    pt[:, qs * 512:(qs + 1) * 512],
                                    start=(kc == 0), stop=(kc == 15),
                                )
                        if qh == 0 and j == 0:
                            pump()
                        if qh == 1 and kc % 4 == 1:
                            pump_cproj()
                    # boundary: pull ya out of PSUM fast, then normalize
                    for e in range(2):
                        nm2 = f"j{j}h{qh}e{e}"
                        yasb = nrm.tile([64, 1024], F32, name=f"ysb_{nm2}", tag="yasb")
                        nc.vector.tensor_copy(yasb[:], ya[e][0:64, :])
                        den = nrm.tile([1, 1024], F32, name=f"den_{nm2}", tag="den")
                        nc.vector.tensor_copy(den[:], ya[e][64:65, :])
                        rden = nrm.tile([1, 1024], F32, name=f"rden_{nm2}", tag="rden")
                        nc.vector.reciprocal_approx_fast(rden[:], den[:])
                        bden = nrm.tile([64, 1024], F32, name=f"bden_{nm2}", tag="bden")
                        nc.gpsimd.partition_broadcast(bden[:], rden[:])
                        nc.vector.tensor_mul(
                            ytf[j][e * 64:e * 64 + 64, qh * 1024:(qh + 1) * 1024],
                            yasb[:], bden[:],
                        )
                if qh == 0:
                    cproj_work.extend(range(8))

            # ---- stage 3: remaining c_proj ----
            while cproj_work:
                pump_cproj()
            for qm in range(8, 16):
                emit_cproj_qm(qm)

    nc.compile()
    return nc


def _qk_perm():
    """f' (0..511) -> within-group feature index (h*64 + d) for q/k.

    f' = half*256 + (h//4)*128 + (h%4)*32 + i maps to d = 2*i + half.
    """
    perm = np.zeros(512, dtype=np.int64)
    for h in range(HL):
        for i in range(32):
            perm[(h // 4) * 128 + (h % 4) * 32 + i] = h * 64 + 2 * i
            perm[256 + (h // 4) * 128 + (h % 4) * 32 + i] = h * 64 + 2 * i + 1
    return perm


def _rope_tables():
    i = np.arange(128) % 32
    inv = (1.0 / (ROPE_THETA ** (np.arange(0, D, 2, dtype=np.float32) / D))).astype(np.float32)
    ang = np.arange(T, dtype=np.float32)[None, :] * inv[i][:, None]
    return np.cos(ang).astype(np.float32), np.sin(ang).astype(np.float32)


def make_in_maps(x, w_attn, w_proj):
    x = np.ascontiguousarray(np.asarray(x, dtype=np.float32))
    w_attn = np.ascontiguousarray(np.asarray(w_attn, dtype=np.float32))
    w_proj = np.ascontiguousarray(np.asarray(w_proj, dtype=np.float32))
    perm = _qk_perm()
    cost, sint = _rope_tables()
    in_maps = []
    xts = [np.ascontiguousarray(x[b].T) for b in range(B)]
    for core in range(N_CORES):
        b, g = core // 2, core % 2
        base = g * 512
        wqc = np.ascontiguousarray(w_attn[base + perm, :].T)
        wkc = np.ascontiguousarray(w_attn[C + base + perm, :].T)
        wvc = np.ascontiguousarray(w_attn[2 * C + base:2 * C + base + 512, :].T)
        wpc = np.ascontiguousarray(w_proj[:, base:base + 512].T)
        in_maps.append(
            {"xt": xts[b], "wq": wqc, "wk": wkc, "wv": wvc, "wp": wpc, "cost": cost, "sint": sint}
        )
    return in_maps


def kernel(x, w_attn, w_proj):
    global _NC
    from concourse.bass_utils import run_bass_kernel_spmd

    if _NC is None:
        _NC = _build_nc()
    in_maps = make_in_maps(x, w_attn, w_proj)
    res = run_bass_kernel_spmd(_NC, in_maps, list(range(N_CORES))).results
    out = np.empty((B, T, C), dtype=np.float32)
    for b in range(B):
        out[b] = res[2 * b]["out"] + res[2 * b + 1]["out"]
    return out
